# revision 22
# baseline (speedup 1.0000x reference)
"""Trainium2 kernel for nn_CorticalColumnLinear.

Computes out[b,s,o] = x[b,s,:] @ (weight*mask)[o,:] with
x [8,4096,1024] f32, weight/mask [1024,1024] f32.

Strategy: pure data-parallel over the batch dim — core i handles x[i]
([4096,1024] @ [1024,1024]^T). The masked weight is replicated.

The mask is 2:4 structured along the INPUT dim: for each group of 4
input columns, 2 are active for ALL output rows.  So (weight*mask)
has only 512 nonzero input columns — the host drops the dead half of
the contraction (and the matching columns of x), halving PE work.

Per-core kernel (trace-tuned to ~73.7 us HW, 2.4x over the 174 us
f32r baseline; measured rel err 3.6e-3 vs the 2e-2 gate):
  - host computes the masked weight, compacts contraction 1024->512,
    pre-transposes x (no PE transposes on device), and casts both
    operands to bf16 (tolerance is 2e-2; bf16 lands ~4e-3).
  - all DMA traffic is packed host-side into partition-major blocks so
    every transfer is 128 contiguous per-partition lines (single
    ~0.6 us trigger, line-merged descriptors): x as 5 ramped
    chunk-blocks [128, kt, mc], w as 2 oc-half blocks [128, kt, 512],
    out as bundled m-tile groups into a [128, 32, 1024] DRAM layout
    the host transposes back.
  - device: everything lives in SBUF; the kernel is a pure matmul
    stream — 256 MMs of N=512 bf16 (4-deep k-accumulation), measured
    at the warm 216 ns/MM roofline.  PE floor 55.3 us; fixed costs
    (NEFF preamble ~7 us, initial weight-DMA gate ~4 us, drain +
    teardown ~5 us) account for the rest.
  - startup: 8 dummy-MM warmup keeps the PE busy through the initial
    DMA fill so the HAM clock-gate is at 8/8 when real MMs start; wA
    arrives in two ring-FIFO halves and the first bundle runs
    partial-k chains so the first real MMs gate on only 256 KB.
  - PSUM evictions (fp32->bf16) alternate scalar/vector engines;
    output bundles alternate the two HWDGE rings; the last two
    bundles shrink (2,1,1) with split evictions + per-oc-half stores
    to cut the drain tail.
"""

import numpy as np
import ml_dtypes

import concourse.mybir as mybir
import concourse.tile as tile
from concourse import bacc
from concourse.bass_utils import run_bass_kernel_spmd

F32 = mybir.dt.float32
BF16 = mybir.dt.bfloat16
BF16NP = np.dtype(ml_dtypes.bfloat16)

B, S, D_IN, D_OUT = 8, 4096, 1024, 1024
P = 128
FD = 512   # matmul moving free dim (one PSUM bank of fp32)

_NC_CACHE = {}


def _chunks(s):
    """x DMA chunk sizes along m: small first so MMs start early."""
    if s >= 4096:
        return [256, 256, 512, 1024, s - 2048]
    out, rem, c = [], s, min(256, s)
    while rem:
        c = min(c, rem)
        out.append(c)
        rem -= c
        c *= 2
    return out


def _bundles(mt_n):
    """Output store bundle sizes (in m-tiles); small at the end."""
    bs, rem = [], mt_n
    while rem > 4:
        bs.append(4)
        rem -= 4
    if rem == 4:
        bs += [2, 1, 1]
    elif rem == 3:
        bs += [2, 1]
    else:
        bs += [1] * rem
    return bs


def build_program(s=S, kc=512):
    kt_n = kc // P
    mt_n = s // P
    chunks = _chunks(s)
    bundles = _bundles(mt_n)

    nc = bacc.Bacc("TRN2", target_bir_lowering=False)
    xbs_d = [
        nc.dram_tensor(f"xb{ci}", [P, kt_n, mc], BF16, kind="ExternalInput")
        for ci, mc in enumerate(chunks)
    ]
    wA_d = nc.dram_tensor("wA", [P, kt_n, FD], BF16, kind="ExternalInput")
    wB_d = nc.dram_tensor("wB", [P, kt_n, FD], BF16, kind="ExternalInput")
    out_d = nc.dram_tensor("out", [P, mt_n, D_OUT], BF16, kind="ExternalOutput")

    with tile.TileContext(nc) as tc:
        with (
            tc.tile_pool(name="wpool", bufs=1) as wpool,
            tc.tile_pool(name="xpool", bufs=1) as xpool,
            tc.tile_pool(name="opool", bufs=6) as opool,
            tc.tile_pool(name="warmp", bufs=1) as warmp,
            tc.tile_pool(name="ps", bufs=8, space="PSUM") as ps,
        ):
            # HAM warmup: junk MMs keep the PE busy (cold, 427 ns each)
            # until wA lands (~11 us); the real matmuls then continue
            # the busy window and the clock gate flips to 8/8 ~3.4 us
            # after the first warmup MM.  Warmup PSUM rotates through
            # the same acc tag so all 8 banks stay available.
            scratch = warmp.tile([P, FD], BF16)
            nc.vector.memset(scratch[:], 0)
            for _ in range(8):
                wps = ps.tile([P, FD], F32, tag="acc")
                nc.tensor.matmul(
                    wps[:], scratch[:, 0:P], scratch[:], start=True, stop=True
                )

            # Weights gate the first accumulation chains: sync ring,
            # first in FIFO order so wA completes before anything else
            # on that ring.
            wts = []
            kh = kt_n // 2
            for name, wd in (("wa", wA_d), ("wb", wB_d)):
                wt_t = wpool.tile([P, kt_n, FD], BF16, name=name)
                if name == "wa" and kh:
                    # wA in two halves (ring FIFO keeps them in order):
                    # the first real MMs gate on just 256 KB.
                    nc.sync.dma_start(wt_t[:, 0:kh, :], wd[:, 0:kh, :])
                    nc.sync.dma_start(wt_t[:, kh:kt_n, :], wd[:, kh:kt_n, :])
                else:
                    nc.sync.dma_start(wt_t[:], wd[:])
                wts.append(wt_t)

            # x: first two (small) chunks trigger from the scalar
            # engine; the bulk chunks trigger from sync (behind the
            # weights) so the scalar engine is free for evictions.
            xts = []
            for ci, mc in enumerate(chunks):
                xt_t = xpool.tile([P, kt_n, mc], BF16, name=f"xt{ci}")
                eng = nc.scalar if ci < 2 else nc.sync
                eng.dma_start(xt_t[:], xbs_d[ci][:])
                xts.append(xt_t)

            # m-tile -> (chunk, local tile) map
            locs = []
            for ci, mc in enumerate(chunks):
                locs += [(ci, j) for j in range(mc // P)]

            def chain(mt, oc, ob, g):
                ci, j = locs[mt]
                acc = ps.tile([P, FD], F32, tag="acc")
                for kt in range(kt_n):
                    nc.tensor.matmul(
                        acc[:],
                        xts[ci][:, kt, j * P:(j + 1) * P],
                        wts[oc][:, kt, :],
                        start=(kt == 0),
                        stop=(kt == kt_n - 1),
                    )
                # scalar+vector can hit PSUM in parallel on different
                # banks; split evictions between them.
                if oc == 0:
                    nc.scalar.copy(ob[:, g, 0:FD], acc[:])
                else:
                    nc.vector.tensor_copy(out=ob[:, g, FD:D_OUT], in_=acc[:])

            def mm_chain(mt, oc, acc):
                ci, j = locs[mt]
                for kt in range(kt_n):
                    nc.tensor.matmul(
                        acc[:],
                        xts[ci][:, kt, j * P:(j + 1) * P],
                        wts[oc][:, kt, :],
                        start=(kt == 0),
                        stop=(kt == kt_n - 1),
                    )

            mt = 0
            for bi, G in enumerate(bundles):
                ob = opool.tile([P, G, D_OUT], BF16, tag="ob")
                last2 = bi >= len(bundles) - 2
                if last2:
                    # Drain-tail path: halve each eviction across
                    # scalar+vector and store each oc half on its own
                    # ring as soon as it is evicted.
                    for g in range(G):
                        for oc in range(2):
                            acc = ps.tile([P, FD], F32, tag="acc")
                            mm_chain(mt + g, oc, acc)
                            lo, hi = oc * FD, oc * FD + FD
                            nc.scalar.copy(ob[:, g, lo:lo + 256], acc[:, 0:256])
                            nc.vector.tensor_copy(
                                out=ob[:, g, lo + 256:hi], in_=acc[:, 256:FD])
                            eng = nc.sync
                            eng.dma_start(
                                out_d[:, mt + g:mt + g + 1, lo:hi],
                                ob[:, g:g + 1, lo:hi])
                elif bi == 0 and kh:
                    # Startup path: wA arrives in halves and wB ~2 us
                    # later.  Run every m-tile's first-half-k partial
                    # chains (gated on wA half 1 only), then the
                    # second-half-k completions, then the oc=1 chains —
                    # the PE never waits for a transfer still in flight.
                    accs = []
                    for g in range(G):
                        ci, j = locs[mt + g]
                        acc = ps.tile([P, FD], F32, tag="acc")
                        for kt in range(kh):
                            nc.tensor.matmul(
                                acc[:],
                                xts[ci][:, kt, j * P:(j + 1) * P],
                                wts[0][:, kt, :],
                                start=(kt == 0), stop=False,
                            )
                        accs.append(acc)
                    for g in range(G):
                        ci, j = locs[mt + g]
                        for kt in range(kh, kt_n):
                            nc.tensor.matmul(
                                accs[g][:],
                                xts[ci][:, kt, j * P:(j + 1) * P],
                                wts[0][:, kt, :],
                                start=False, stop=(kt == kt_n - 1),
                            )
                        nc.scalar.copy(ob[:, g, 0:FD], accs[g][:])
                    for g in range(G):
                        chain(mt + g, 1, ob, g)
                else:
                    for g in range(G):
                        chain(mt + g, 0, ob, g)
                        chain(mt + g, 1, ob, g)
                mt += G
                if not last2:
                    eng = nc.sync
                    eng.dma_start(out_d[:, mt - G:mt, :], ob[:])

    nc.finalize()
    return nc


def _get_program(s, kc):
    key = (s, kc)
    if key not in _NC_CACHE:
        _NC_CACHE[key] = build_program(s, kc)
    return _NC_CACHE[key]


def _prep(x, weight, mask):
    """Host prep: mask, compact dead input columns, transpose, pack
    partition-major DMA blocks, bf16-cast.  Returns per-core in_maps."""
    x = np.asarray(x, dtype=np.float32)
    weight = np.asarray(weight, dtype=np.float32)
    mask = np.asarray(mask, dtype=np.float32)
    s = x.shape[1]

    w = weight * mask                        # exact elementwise product
    act = np.flatnonzero(mask.any(axis=0))   # live input columns
    kc = len(act)
    kcp = max(P, -(-kc // P) * P)            # pad to multiple of 128
    kt_n = kcp // P

    wtc = np.zeros((kcp, D_OUT), dtype=np.float32)
    wtc[:kc] = w[:, act].T
    wA = np.ascontiguousarray(
        wtc[:, :FD].reshape(kt_n, P, FD).transpose(1, 0, 2).astype(BF16NP))
    wB = np.ascontiguousarray(
        wtc[:, FD:].reshape(kt_n, P, FD).transpose(1, 0, 2).astype(BF16NP))

    chunks = _chunks(s)
    in_maps = []
    for i in range(x.shape[0]):
        xtp = np.zeros((kcp, s), dtype=np.float32)
        xtp[:kc] = x[i].T[act]
        m = {"wA": wA, "wB": wB}
        off = 0
        for ci, mc in enumerate(chunks):
            m[f"xb{ci}"] = np.ascontiguousarray(
                xtp[:, off:off + mc].reshape(kt_n, P, mc)
                .transpose(1, 0, 2).astype(BF16NP))
            off += mc
        in_maps.append(m)
    return in_maps, s, kcp


def run(x, weight, mask, trace=False):
    in_maps, s, kcp = _prep(x, weight, mask)
    nc = _get_program(s, kcp)
    res = run_bass_kernel_spmd(nc, in_maps, list(range(len(in_maps))), trace=trace)
    out = np.stack(
        [
            np.asarray(res.results[i]["out"])
            .transpose(1, 0, 2).reshape(s, D_OUT).astype(np.float32)
            for i in range(len(in_maps))
        ],
        axis=0,
    )
    return out, res


def kernel(x, weight, mask):
    out, _ = run(x, weight, mask)
    return out
